# revision 8
# baseline (speedup 1.0000x reference)
"""Trainium2 Bass kernel for a 2-layer GATv2 + global mean pool (GNN message passing).

Strategy (8 NeuronCores, SPMD):
  - Host: sort edges by target node; partition nodes into 8 equal contiguous
    ranges; each core owns all in-edges of its node range, so softmax
    segment-reductions are fully core-local.
  - Edges are grouped into 128-node windows; each window's edge list is padded
    to a fixed number of 128-edge subtiles (T) so the kernel is static.
  - Layer 1: input features are tiny (2-d), so per-edge features are shipped as
    a 7-row "cat" matrix ([x_src; x_tgt; edge_attr; 1]) and all node/edge
    transforms become small matmuls.  |att| is folded into the weights so the
    logit head-reduction is a matmul too (signs in an R matrix); segment-sums
    are one-hot matmuls accumulated in PSUM (edge-sorted => local windows).
  - Between layers: AllGather of the per-node xl2 table (h @ W_l2).
  - Layer 2: gather xl2[src] / xr2[tgt] rows by indirect DMA, logits via
    vector ops, same one-hot segment-sum, then a one-hot matmul vs graph ids
    for global pooling.  Host sums the 8 per-core pooled partials and divides
    by graph sizes.
"""
import math
import numpy as np
import ml_dtypes

import concourse.bass as bass
import concourse.tile as tile
from concourse import bacc, mybir
from concourse.bass_utils import run_bass_kernel_spmd

F32 = mybir.dt.float32
F32R = mybir.dt.float32r
BF16 = mybir.dt.bfloat16
I32 = mybir.dt.int32
BF16NP = ml_dtypes.bfloat16

N, E, G = 50000, 800000, 64
NCORES = 8
NLOC = N // NCORES            # 6250 nodes per core
WINP = 128                    # nodes per window
NW = (NLOC + WINP - 1) // WINP  # 49 windows
H1, C1, D1 = 16, 8, 128
H2, C2, D2 = 4, 16, 64
LRELU_ALPHA = 0.2
PAD_SENTINEL = -5.0


# --------------------------------------------------------------------------
# Host-side preprocessing
# --------------------------------------------------------------------------
def _prep(inputs):
    x = np.ascontiguousarray(np.asarray(inputs["x"], dtype=np.float32))
    ea = np.ascontiguousarray(np.asarray(inputs["edge_attr"], dtype=np.float32))
    ei = np.asarray(inputs["edge_index"])
    batch = np.asarray(inputs["batch"]).astype(np.int64)
    src = ei[0].astype(np.int64)
    tgt = ei[1].astype(np.int64)

    order = np.argsort(tgt, kind="stable")
    src_s = src[order]
    tgt_s = tgt[order]
    ea_s = ea[order]

    # per-(core,window) edge counts -> T (subtiles per window, even)
    T = 1
    seg = []
    for d in range(NCORES):
        lo, hi = np.searchsorted(tgt_s, [d * NLOC, (d + 1) * NLOC])
        ltgt = tgt_s[lo:hi] - d * NLOC
        w = ltgt >> 7
        cnt = np.bincount(w, minlength=NW)
        T = max(T, int(math.ceil(cnt.max() / WINP)))
        seg.append((lo, hi))
    T += T & 1  # even (chunks of 256)
    EPW = T * WINP          # edge slots per window
    NSUB = NW * T           # subtiles per core
    EP = NW * EPW           # edge slots per core

    # constants (shared by all cores)
    f32 = np.float32
    W_l1 = np.asarray(inputs["W_l1"], f32); b_l1 = np.asarray(inputs["b_l1"], f32)
    W_r1 = np.asarray(inputs["W_r1"], f32); b_r1 = np.asarray(inputs["b_r1"], f32)
    W_e1 = np.asarray(inputs["W_e1"], f32)
    att1 = np.asarray(inputs["att1"], f32)
    bias1 = np.asarray(inputs["bias1"], f32)
    W_l2 = np.asarray(inputs["W_l2"], f32); b_l2 = np.asarray(inputs["b_l2"], f32)
    W_r2 = np.asarray(inputs["W_r2"], f32); b_r2 = np.asarray(inputs["b_r2"], f32)
    W_e2 = np.asarray(inputs["W_e2"], f32)
    att2 = np.asarray(inputs["att2"], f32)
    bias2 = np.asarray(inputs["bias2"], f32)

    att1f = att1.reshape(D1)
    wcats = np.concatenate(
        [W_l1, W_r1, W_e1, (b_l1 + b_r1)[None, :]], axis=0
    ) * np.abs(att1f)[None, :]                                   # [7,128]
    rsign = np.zeros((D1, H1), f32)
    rsign[np.arange(D1), np.arange(D1) // C1] = np.sign(att1f)    # [128,16]
    wl1aug = np.concatenate(
        [W_l1, np.zeros((4, D1), f32), b_l1[None, :]], axis=0
    ).astype(BF16NP)                                              # [7,128]
    we2 = W_e2.astype(BF16NP)                                     # [2,64]
    w2cat = np.concatenate([W_l2, W_r2], axis=1)                  # [128,128]
    b2rep = np.tile(np.concatenate([b_l2, b_r2])[None, :], (WINP, 1))
    att2rep = np.tile(att2.reshape(D2)[None, :], (WINP, 1))       # [128,64]
    bias1rep = np.tile(bias1[None, :], (WINP, 1))                 # [128,128]
    bias2rep = np.tile(bias2[None, :], (WINP, 1))                 # [128,64]
    iota128 = np.tile(np.arange(WINP, dtype=f32)[None, :], (WINP, 1))
    iota64 = np.tile(np.arange(G, dtype=f32)[None, :], (WINP, 1))
    ident = np.eye(WINP, dtype=f32)

    consts = dict(
        wcats=wcats.astype(BF16NP), rsign=rsign.astype(BF16NP), wl1aug=wl1aug, we2=we2,
        w2cat=w2cat.astype(f32), b2rep=b2rep.astype(f32),
        att2rep=att2rep.astype(f32), bias1rep=bias1rep.astype(f32),
        bias2rep=bias2rep.astype(f32), iota128=iota128, iota64=iota64,
        ident=ident,
    )

    in_maps = []
    for d in range(NCORES):
        lo, hi = seg[d]
        ssrc = src_s[lo:hi]
        ltgt = tgt_s[lo:hi] - d * NLOC
        sea = ea_s[lo:hi]
        w = ltgt >> 7
        wstart = np.searchsorted(w, np.arange(NW))
        rank = np.arange(len(w)) - wstart[w]
        g = w * EPW + rank                                       # global slot

        catf = np.zeros((7, EP), f32)
        catf[0:2, g] = x[ssrc].T
        catf[2:4, g] = x[ltgt + d * NLOC].T
        catf[4:6, g] = sea.T
        catf[6, g] = 1.0
        cat16 = np.ascontiguousarray(
            catf.reshape(7, NW, EPW).transpose(1, 0, 2)).astype(BF16NP)  # [NW,7,EPW]

        tsh = np.full(EP, PAD_SENTINEL, f32)
        tsh[g] = (ltgt - w * WINP).astype(f32)
        tshift = np.ascontiguousarray(tsh.reshape(NSUB, WINP).T)  # [128,NSUB]

        sidx = np.zeros(EP, np.int32)
        sidx[g] = ssrc.astype(np.int32)
        srcidx = np.ascontiguousarray(sidx.reshape(NSUB, WINP).T)

        tidx = np.zeros(EP, np.int32)
        tidx[g] = ltgt.astype(np.int32)
        tgtidx = np.ascontiguousarray(tidx.reshape(NSUB, WINP).T)

        batchw = np.full((WINP, NW), PAD_SENTINEL, f32)
        nodes = np.arange(NLOC)
        batchw[nodes % WINP, nodes // WINP] = batch[d * NLOC + nodes].astype(f32)

        m = dict(cat16=cat16, tshift=tshift, srcidx=srcidx,
                 tgtidx=tgtidx, batchw=batchw)
        m.update(consts)
        in_maps.append(m)

    counts = np.bincount(batch, minlength=G).astype(np.float32)
    return in_maps, T, counts


# --------------------------------------------------------------------------
# Device program
# --------------------------------------------------------------------------
DEBUG = False

def _build(T):
    EPW = T * WINP
    NSUB = NW * T
    CH = T // 2  # 256-edge chunks per window

    nc = bacc.Bacc("TRN2", target_bir_lowering=False, debug=False,
                   num_devices=NCORES, enable_asserts=False)

    def din(name, shape, dt):
        return nc.dram_tensor(name, shape, dt, kind="ExternalInput").ap()

    cat16 = din("cat16", [NW, 7, EPW], BF16)
    tshift = din("tshift", [WINP, NSUB], F32)
    srcidx = din("srcidx", [WINP, NSUB], I32)
    tgtidx = din("tgtidx", [WINP, NSUB], I32)
    batchw = din("batchw", [WINP, NW], F32)
    wcats = din("wcats", [7, D1], BF16)
    rsign = din("rsign", [D1, H1], BF16)
    wl1aug = din("wl1aug", [7, D1], BF16)
    we2 = din("we2", [2, D2], BF16)
    w2cat = din("w2cat", [D1, D1], F32)
    b2rep = din("b2rep", [WINP, D1], F32)
    att2rep = din("att2rep", [WINP, D2], F32)
    bias1rep = din("bias1rep", [WINP, D1], F32)
    bias2rep = din("bias2rep", [WINP, D2], F32)
    iota128 = din("iota128", [WINP, WINP], F32)
    iota64 = din("iota64", [WINP, G], F32)
    ident = din("ident", [WINP, WINP], F32)

    pool_out = nc.dram_tensor("pool_out", [G, D2], F32, kind="ExternalOutput").ap()
    if DEBUG:
        dbg_h = nc.dram_tensor("dbg_h", [NW * WINP, D1], F32, kind="ExternalOutput").ap()
        dbg_h2 = nc.dram_tensor("dbg_h2", [NW * WINP, D2], F32, kind="ExternalOutput").ap()
        dbg_den = nc.dram_tensor("dbg_den", [NW * WINP, H1], F32, kind="ExternalOutput").ap()
        dbg_lr = nc.dram_tensor("dbg_lr", [D1, EPW], F32, kind="ExternalOutput").ap()
        dbg_ee = nc.dram_tensor("dbg_ee", [WINP, 2 * H1 * (T // 2)], F32, kind="ExternalOutput").ap()

    xl2loc = nc.dram_tensor("xl2loc", [NLOC, D2], F32).ap()
    xr2loc = nc.dram_tensor("xr2loc", [NLOC, D2], F32).ap()
    xl2full = nc.dram_tensor("xl2full", [N, D2], F32, addr_space="Shared").ap()

    with tile.TileContext(nc) as tc:
        with tc.tile_pool(name="const", bufs=1) as cpool, \
             tc.tile_pool(name="idx", bufs=1) as ipool, \
             tc.tile_pool(name="catw", bufs=3) as catpool, \
             tc.tile_pool(name="work", bufs=3) as wpool, \
             tc.tile_pool(name="acc", bufs=2, space="PSUM") as psA, \
             tc.tile_pool(name="big", bufs=2, space="PSUM") as psB:

            def cload(ap, shape, dt, tag):
                t = cpool.tile(shape, dt, tag=tag)
                nc.sync.dma_start(out=t[:], in_=ap[:, :])
                return t

            wcats_sb = cload(wcats, [7, D1], BF16, "k_wcats")
            rsign_sb = cload(rsign, [D1, H1], BF16, "k_rsign")
            wl1_sb = cload(wl1aug, [7, D1], BF16, "k_wl1")
            we2_sb = cload(we2, [2, D2], BF16, "k_we2")
            w2cat_sb = cload(w2cat, [D1, D1], F32, "k_w2cat")
            b2rep_sb = cload(b2rep, [WINP, D1], F32, "k_b2rep")
            att2rep_sb = cload(att2rep, [WINP, D2], F32, "k_att2rep")
            bias1rep_sb = cload(bias1rep, [WINP, D1], F32, "k_bias1rep")
            bias2rep_sb = cload(bias2rep, [WINP, D2], F32, "k_bias2rep")
            iota128_sb = cload(iota128, [WINP, WINP], F32, "k_iota128")
            iota64_sb = cload(iota64, [WINP, G], F32, "k_iota64")
            ident_sb = cload(ident, [WINP, WINP], F32, "k_ident")

            tshift_sb = ipool.tile([WINP, NSUB], F32)
            nc.sync.dma_start(out=tshift_sb[:], in_=tshift[:, :])
            srcidx_sb = ipool.tile([WINP, NSUB], I32)
            nc.sync.dma_start(out=srcidx_sb[:], in_=srcidx[:, :])
            tgtidx_sb = ipool.tile([WINP, NSUB], I32)
            nc.sync.dma_start(out=tgtidx_sb[:], in_=tgtidx[:, :])
            batchw_sb = ipool.tile([WINP, NW], F32)
            nc.sync.dma_start(out=batchw_sb[:], in_=batchw[:, :])

            # ------------------------------------------------ Phase A: layer 1
            with tc.tile_pool(name="ps_small", bufs=2, space="PSUM") as psC, \
                 tc.tile_pool(name="ps_xl", bufs=2, space="PSUM") as psD:
                for w in range(NW):
                    c16w = catpool.tile([7, EPW], BF16, tag="c16w")
                    nc.sync.dma_start(out=c16w[:], in_=cat16[w])

                    acc = psA.tile([WINP, D1 + H1], F32, tag="acc")
                    for c in range(CH):
                        off = c * 256
                        u_ps = psB.tile([D1, 256], F32, tag="u")
                        nc.tensor.matmul(out=u_ps[:], lhsT=wcats_sb[:],
                                         rhs=c16w[:, off:off + 256],
                                         start=True, stop=True)
                        lrelu_u = wpool.tile([D1, 256], BF16, tag="lrelu")
                        nc.scalar.activation(out=lrelu_u[:], in_=u_ps[:],
                                             func=mybir.ActivationFunctionType.Prelu,
                                             alpha=LRELU_ALPHA)
                        et = psC.tile([WINP, 256 + 2 * H1], F32, tag="eps")
                        nc.tensor.matmul(out=et[:H1, 0:256], lhsT=rsign_sb[:],
                                         rhs=lrelu_u[:], start=True, stop=True)
                        e_sb = wpool.tile([H1, 256], F32, tag="esb")
                        nc.scalar.activation(out=e_sb[:], in_=et[:H1, 0:256],
                                             func=mybir.ActivationFunctionType.Copy)
                        for k in range(2):
                            nc.tensor.transpose(
                                out=et[:, 256 + k * H1:256 + (k + 1) * H1],
                                in_=e_sb[:, k * 128:(k + 1) * 128],
                                identity=ident_sb[:H1, :H1])
                        ee_sb = wpool.tile([WINP, 2 * H1], BF16, tag="eesb")
                        nc.scalar.activation(out=ee_sb[:], in_=et[:, 256:256 + 2 * H1],
                                             func=mybir.ActivationFunctionType.Exp)
                        if DEBUG and w == 0:
                            nc.gpsimd.dma_start(out=dbg_lr[:, off:off + 256],
                                                in_=lrelu_u[:])
                            nc.gpsimd.dma_start(
                                out=dbg_ee[:, c * 2 * H1:(c + 1) * 2 * H1],
                                in_=ee_sb[:])
                        for s in range(2):
                            su = w * T + c * 2 + s
                            so = off + s * 128
                            xl_ps = psD.tile([WINP, D1], F32, tag="xl")
                            nc.tensor.matmul(out=xl_ps[:],
                                             lhsT=c16w[:, so:so + 128],
                                             rhs=wl1_sb[:], start=True, stop=True)
                            rhs_sb = wpool.tile([WINP, D1 + H1], BF16, tag="rhs1")
                            nc.vector.tensor_tensor(
                                out=rhs_sb[:, 0:D1].rearrange("p (h c) -> p h c", c=C1),
                                in0=xl_ps[:].rearrange("p (h c) -> p h c", c=C1),
                                in1=ee_sb[:, s * H1:(s + 1) * H1]
                                    .unsqueeze(2).broadcast_to([WINP, H1, C1]),
                                op=mybir.AluOpType.mult)
                            nc.scalar.activation(
                                out=rhs_sb[:, D1:D1 + H1],
                                in_=ee_sb[:, s * H1:(s + 1) * H1],
                                func=mybir.ActivationFunctionType.Copy)
                            s_sb = wpool.tile([WINP, WINP], BF16, tag="ssb")
                            nc.vector.tensor_tensor(
                                out=s_sb[:],
                                in0=tshift_sb[:, su:su + 1].to_broadcast([WINP, WINP]),
                                in1=iota128_sb[:],
                                op=mybir.AluOpType.is_equal)
                            nc.tensor.matmul(out=acc[:], lhsT=s_sb[:], rhs=rhs_sb[:],
                                             start=(c == 0 and s == 0),
                                             stop=(c == CH - 1 and s == 1))

                    # window epilogue: h = Num/Den + bias1, ELU, xl2/xr2 tables
                    den_sb = wpool.tile([WINP, H1], F32, tag="den")
                    nc.vector.tensor_scalar_max(den_sb[:], acc[:, D1:D1 + H1], 1e-30)
                    rec_sb = wpool.tile([WINP, H1], F32, tag="rec")
                    nc.vector.reciprocal(rec_sb[:], den_sb[:])
                    h_sb = wpool.tile([WINP, D1], F32, tag="hsb")
                    nc.vector.tensor_tensor(
                        out=h_sb[:].rearrange("p (h c) -> p h c", c=C1),
                        in0=acc[:, 0:D1].rearrange("p (h c) -> p h c", c=C1),
                        in1=rec_sb[:].unsqueeze(2).broadcast_to([WINP, H1, C1]),
                        op=mybir.AluOpType.mult)
                    hb_sb = wpool.tile([WINP, D1], F32, tag="hbsb")
                    nc.vector.tensor_add(hb_sb[:], h_sb[:], bias1rep_sb[:])
                    # ELU = relu(h) + exp(min(h,0)) - 1
                    neg_sb = wpool.tile([WINP, D1], F32, tag="negsb")
                    nc.vector.tensor_scalar_min(neg_sb[:], hb_sb[:], 0.0)
                    exp_sb = wpool.tile([WINP, D1], F32, tag="expsb")
                    nc.scalar.activation(out=exp_sb[:], in_=neg_sb[:],
                                         func=mybir.ActivationFunctionType.Exp)
                    pos_sb = wpool.tile([WINP, D1], F32, tag="possb")
                    nc.vector.tensor_scalar_max(pos_sb[:], hb_sb[:], 0.0)
                    helu = wpool.tile([WINP, D1], F32, tag="helu")
                    nc.vector.tensor_add(helu[:], pos_sb[:], exp_sb[:])
                    nc.vector.tensor_scalar_add(helu[:], helu[:], -1.0)

                    if DEBUG:
                        nc.sync.dma_start(out=dbg_h[w * WINP:(w + 1) * WINP, :],
                                          in_=helu[:])
                        nc.sync.dma_start(out=dbg_den[w * WINP:(w + 1) * WINP, :],
                                          in_=den_sb[:])
                    ht_ps = psC.tile([WINP, 256 + 2 * H1], F32, tag="eps")
                    nc.tensor.transpose(out=ht_ps[:, 0:WINP], in_=helu[:], identity=ident_sb[:])
                    ht_sb = wpool.tile([D1, WINP], F32, tag="htsb")
                    nc.scalar.activation(out=ht_sb[:], in_=ht_ps[:, 0:WINP],
                                         func=mybir.ActivationFunctionType.Copy)
                    x2_ps = psC.tile([WINP, 256 + 2 * H1], F32, tag="eps")
                    nc.tensor.matmul(out=x2_ps[:, 0:D1], lhsT=ht_sb[:], rhs=w2cat_sb[:],
                                     start=True, stop=True)
                    x2_sb = wpool.tile([WINP, D1], F32, tag="x2sb")
                    nc.vector.tensor_add(x2_sb[:], x2_ps[:, 0:D1], b2rep_sb[:])
                    rows = min(WINP, NLOC - w * WINP)
                    nc.sync.dma_start(out=xl2loc[w * WINP:w * WINP + rows, :],
                                      in_=x2_sb[:rows, 0:D2])
                    nc.sync.dma_start(out=xr2loc[w * WINP:w * WINP + rows, :],
                                      in_=x2_sb[:rows, D2:D1])

            # -------------------------------------------- AllGather xl2 table
            nc.gpsimd.collective_compute(
                "AllGather", mybir.AluOpType.bypass,
                ins=[xl2loc[:, :]], outs=[xl2full[:, :]],
                replica_groups=[list(range(NCORES))])

            # ------------------------------------------------ Phase B: layer 2
            with tc.tile_pool(name="ps_pool", bufs=1, space="PSUM") as psP, \
                 tc.tile_pool(name="gath", bufs=4) as gpool:
                pool_ps = psP.tile([G, D2], F32)
                for w in range(NW):
                    c16b = catpool.tile([2, EPW], BF16, tag="c16b")
                    nc.sync.dma_start(out=c16b[:], in_=cat16[w, 4:6, :])
                    acc2 = psA.tile([WINP, D2 + H2], F32, tag="acc")
                    for t in range(T):
                        su = w * T + t
                        so = t * 128
                        xg = gpool.tile([WINP, D2], F32, tag="xg")
                        nc.gpsimd.indirect_dma_start(
                            out=xg[:], out_offset=None,
                            in_=xl2full[:, :],
                            in_offset=bass.IndirectOffsetOnAxis(
                                ap=srcidx_sb[:, su:su + 1], axis=0))
                        xr = gpool.tile([WINP, D2], F32, tag="xr")
                        nc.gpsimd.indirect_dma_start(
                            out=xr[:], out_offset=None,
                            in_=xr2loc[:, :],
                            in_offset=bass.IndirectOffsetOnAxis(
                                ap=tgtidx_sb[:, su:su + 1], axis=0))
                        xe_ps = psB.tile([WINP, D2], F32, tag="u")
                        nc.tensor.matmul(out=xe_ps[:], lhsT=c16b[:, so:so + 128],
                                         rhs=we2_sb[:], start=True, stop=True)
                        m2a = wpool.tile([WINP, D2], F32, tag="m2a")
                        nc.vector.tensor_add(m2a[:], xg[:], xr[:])
                        m2 = wpool.tile([WINP, D2], F32, tag="m2")
                        nc.vector.tensor_add(m2[:], m2a[:], xe_ps[:])
                        lr2 = wpool.tile([WINP, D2], F32, tag="lr2")
                        nc.scalar.activation(out=lr2[:], in_=m2[:],
                                             func=mybir.ActivationFunctionType.Prelu,
                                             alpha=LRELU_ALPHA)
                        ta = wpool.tile([WINP, D2], F32, tag="ta")
                        nc.vector.tensor_mul(ta[:], lr2[:], att2rep_sb[:])
                        e2 = wpool.tile([WINP, H2], F32, tag="e2")
                        nc.vector.tensor_reduce(
                            out=e2[:], in_=ta[:].rearrange("p (h c) -> p h c", c=C2),
                            axis=mybir.AxisListType.X, op=mybir.AluOpType.add)
                        ee2 = wpool.tile([WINP, H2], BF16, tag="ee2")
                        nc.scalar.activation(out=ee2[:], in_=e2[:],
                                             func=mybir.ActivationFunctionType.Exp)
                        rhs2 = wpool.tile([WINP, D2 + H2], BF16, tag="rhs2")
                        nc.vector.tensor_tensor(
                            out=rhs2[:, 0:D2].rearrange("p (h c) -> p h c", c=C2),
                            in0=xg[:].rearrange("p (h c) -> p h c", c=C2),
                            in1=ee2[:].unsqueeze(2).broadcast_to([WINP, H2, C2]),
                            op=mybir.AluOpType.mult)
                        nc.scalar.activation(
                            out=rhs2[:, D2:D2 + H2], in_=ee2[:],
                            func=mybir.ActivationFunctionType.Copy)
                        s_sb = wpool.tile([WINP, WINP], BF16, tag="ssb")
                        nc.vector.tensor_tensor(
                            out=s_sb[:],
                            in0=tshift_sb[:, su:su + 1].to_broadcast([WINP, WINP]),
                            in1=iota128_sb[:],
                            op=mybir.AluOpType.is_equal)
                        nc.tensor.matmul(out=acc2[:], lhsT=s_sb[:], rhs=rhs2[:],
                                         start=(t == 0), stop=(t == T - 1))

                    den2 = wpool.tile([WINP, H2], F32, tag="den")
                    nc.vector.tensor_scalar_max(den2[:], acc2[:, D2:D2 + H2], 1e-30)
                    rec2 = wpool.tile([WINP, H2], F32, tag="rec")
                    nc.vector.reciprocal(rec2[:], den2[:])
                    h2_sb = wpool.tile([WINP, D2], F32, tag="h2sb")
                    nc.vector.tensor_tensor(
                        out=h2_sb[:].rearrange("p (h c) -> p h c", c=C2),
                        in0=acc2[:, 0:D2].rearrange("p (h c) -> p h c", c=C2),
                        in1=rec2[:].unsqueeze(2).broadcast_to([WINP, H2, C2]),
                        op=mybir.AluOpType.mult)
                    nc.vector.tensor_add(h2_sb[:], h2_sb[:], bias2rep_sb[:])
                    if DEBUG:
                        nc.sync.dma_start(out=dbg_h2[w * WINP:(w + 1) * WINP, :],
                                          in_=h2_sb[:])
                    b_sb = wpool.tile([WINP, G], F32, tag="bsb")
                    nc.vector.tensor_tensor(
                        out=b_sb[:],
                        in0=batchw_sb[:, w:w + 1].to_broadcast([WINP, G]),
                        in1=iota64_sb[:],
                        op=mybir.AluOpType.is_equal)
                    nc.tensor.matmul(out=pool_ps[:], lhsT=b_sb[:], rhs=h2_sb[:],
                                     start=(w == 0), stop=(w == NW - 1))

                pool_sb = wpool.tile([G, D2], F32, tag="poolsb")
                nc.vector.tensor_copy(pool_sb[:], pool_ps[:])
                nc.sync.dma_start(out=pool_out[:, :], in_=pool_sb[:])

    nc.compile()
    return nc


_CACHE = {}


def kernel(**inputs):
    in_maps, T, counts = _prep(inputs)
    if T not in _CACHE:
        _CACHE[T] = _build(T)
    nc = _CACHE[T]
    res = run_bass_kernel_spmd(nc, in_maps, core_ids=list(range(NCORES)))
    pool = np.zeros((G, D2), np.float64)
    for d in range(NCORES):
        pool += res.results[d]["pool_out"].astype(np.float64)
    out = pool / np.maximum(counts, 1.0)[:, None]
    return out.astype(np.float32)
